# revision 1
# baseline (speedup 1.0000x reference)
"""NT-Xent loss on 8 TRN2 NeuronCores.

Reference computes, for z = concat(z1, z2) (2N=8192 rows, D=256):
    zn  = z / max(||z||, eps)
    sim = (zn @ zn.T) / T, diag masked to -1e9
    loss = mean_i( logsumexp_j sim[i, j] - sim[i, pos(i)] ),  pos(i) = (i + N) % 2N

Sharding: 2N rows split into 8 blocks of 1024. Each core computes its
1024x8192 row-block of sim against the full replicated zn.T, fused with
exp+rowsum on ScalarE (accum_out), so the sim matrix is never
materialized in HBM.

SPMD trick: core c receives zn.T with columns rotated left by c*1024, so
on EVERY core its own rows sit at columns 0:1024 and the positive
partners at columns 4096:5120. All diagonal-window access patterns are
then compile-time constants, identical across cores; only the data
differs. The exp'd self-similarity and positive-pair diagonals are
extracted from the ScalarE output tile with an eye-mask multiply +
reduce on VectorE; the host subtracts exp(self/T) from the denominator
sum and computes log() and the final mean (tiny).
"""

import sys

if "/opt/trn_rl_repo" not in sys.path:
    sys.path.insert(0, "/opt/trn_rl_repo")

import ml_dtypes
import numpy as np

import concourse.bass as bass
import concourse.mybir as mybir
import concourse.tile as tile
from concourse import bacc
from concourse.bass_utils import run_bass_kernel_spmd

N = 4096
D = 256
TWO_N = 2 * N          # 8192
TEMPERATURE = 0.07
EPS = 1e-8
N_CORES = 8
ROWS_PER_CORE = TWO_N // N_CORES   # 1024
M_TILES = ROWS_PER_CORE // 128     # 8 row-tiles of 128
CB = 2048                          # psum / column-block width
N_CB = TWO_N // CB                 # 4 column blocks
POS_CB = N // CB                   # column block holding the positives (2)

_cached = {}


def _build_bass(m_tiles=M_TILES):
    f32 = mybir.dt.float32
    bf16 = mybir.dt.bfloat16
    nc = bacc.Bacc("TRN2", target_bir_lowering=False, debug=False)

    znt = nc.declare_dram_parameter("znt", [D, TWO_N], bf16, isOutput=False)
    eye = nc.declare_dram_parameter("eye", [128, 128], f32, isOutput=False)
    s_out = nc.declare_dram_parameter("S", [128, m_tiles], f32, isOutput=True)
    sexp_out = nc.declare_dram_parameter("sexp", [128, m_tiles], f32, isOutput=True)
    pexp_out = nc.declare_dram_parameter("pexp", [128, m_tiles], f32, isOutput=True)

    with tile.TileContext(nc) as tc:
        with (
            tc.tile_pool(name="zchunks", bufs=1) as zpool,
            tc.tile_pool(name="consts", bufs=1) as cpool,
            tc.tile_pool(name="stats", bufs=1) as spool,
            tc.tile_pool(name="discard", bufs=4) as dpool,
            tc.tile_pool(name="scratch", bufs=2) as scpool,
            tc.tile_pool(name="psum", bufs=2, space=bass.MemorySpace.PSUM) as ppool,
        ):
            # Column-chunked copies of znt: zt[k][cb] holds rows k*128:(k+1)*128,
            # cols cb*2048:(cb+1)*2048. Separate tiles => independent DMA deps,
            # so phase cb only waits on its own chunks.
            zt = [[None] * N_CB for _ in range(2)]
            for cb in range(N_CB):
                for k in range(2):
                    t = zpool.tile([128, CB], bf16, tag=f"z{k}_{cb}")
                    # alternate DMA issue engines so descriptor issue isn't
                    # serialized on one queue (head-latency win)
                    eng = nc.sync if (cb * 2 + k) % 2 == 0 else nc.gpsimd
                    eng.dma_start(t[:], znt[k * 128 : (k + 1) * 128, cb * CB : (cb + 1) * CB])
                    zt[k][cb] = t

            eye_t = cpool.tile([128, 128], f32, tag="eye")
            nc.sync.dma_start(eye_t[:], eye[:])

            acc = spool.tile([128, m_tiles * N_CB], f32, tag="acc")
            s_t = spool.tile([128, m_tiles], f32, tag="S")
            sexp_t = spool.tile([128, m_tiles], f32, tag="sexp")
            pexp_t = spool.tile([128, m_tiles], f32, tag="pexp")

            for cb in range(N_CB):
                for m in range(m_tiles):
                    moff = m * 128
                    ps = ppool.tile([128, CB], f32, tag="ps")
                    for k in range(2):
                        for nn in range(CB // 512):
                            nc.tensor.matmul(
                                ps[:, nn * 512 : (nn + 1) * 512],
                                lhsT=zt[k][0][:, moff : moff + 128],
                                rhs=zt[k][cb][:, nn * 512 : (nn + 1) * 512],
                                start=(k == 0),
                                stop=(k == 1),
                            )
                    ex = dpool.tile([128, CB], f32, tag="ex")
                    nc.scalar.activation(
                        out=ex[:],
                        in_=ps[:],
                        func=mybir.ActivationFunctionType.Exp,
                        bias=0.0,
                        scale=1.0 / TEMPERATURE,
                        accum_out=acc[:, m * N_CB + cb : m * N_CB + cb + 1],
                    )
                    # extract exp'd diagonals from the SBUF exp tile:
                    # self-sim diag lives in cb 0 at cols moff:moff+128,
                    # positive-pair diag in cb POS_CB at the same offset.
                    for cond, dst in ((cb == 0, sexp_t), (cb == POS_CB, pexp_t)):
                        if cond:
                            poff = moff  # 4096 % CB == 0: same offset in cb 0 and cb 2
                            sc = scpool.tile([128, 128], f32, tag="sc")
                            nc.vector.tensor_tensor(
                                sc[:],
                                ex[:, poff : poff + 128],
                                eye_t[:],
                                mybir.AluOpType.mult,
                            )
                            nc.vector.reduce_sum(
                                dst[:, m : m + 1], sc[:], axis=mybir.AxisListType.X
                            )
                    if cb == N_CB - 1:
                        # final rowsum for this row-tile as soon as its last
                        # column block is done — overlaps the kernel tail
                        nc.vector.reduce_sum(
                            s_t[:, m : m + 1],
                            acc[:, m * N_CB : (m + 1) * N_CB],
                            axis=mybir.AxisListType.X,
                        )

            nc.sync.dma_start(s_out[:], s_t[:])
            nc.sync.dma_start(sexp_out[:], sexp_t[:])
            nc.sync.dma_start(pexp_out[:], pexp_t[:])

    nc.compile()
    return nc


def _prepare_inputs(z1, z2):
    z = np.concatenate([np.asarray(z1), np.asarray(z2)], axis=0).astype(np.float32)
    norms = np.maximum(np.sqrt((z.astype(np.float64) ** 2).sum(-1)), EPS)
    zn = (z / norms[:, None]).astype(np.float32)
    znb = zn.astype(ml_dtypes.bfloat16)
    znt = np.ascontiguousarray(znb.T)  # [D, 2N]
    eye = np.eye(128, dtype=np.float32)
    in_maps = []
    for c in range(N_CORES):
        znt_c = np.ascontiguousarray(np.roll(znt, -c * ROWS_PER_CORE, axis=1))
        in_maps.append({"znt": znt_c, "eye": eye})
    return in_maps


def kernel(z1, z2):
    if "nc" not in _cached:
        _cached["nc"] = _build_bass()
    nc = _cached["nc"]
    in_maps = _prepare_inputs(z1, z2)
    res = run_bass_kernel_spmd(nc, in_maps, core_ids=list(range(N_CORES)))
    results = res.results

    per_row_loss = np.zeros(TWO_N, dtype=np.float64)
    for c in range(N_CORES):
        # [128, M_TILES]; element [l, m] belongs to global row c*1024 + m*128 + l
        S = np.asarray(results[c]["S"], dtype=np.float64)
        sexp = np.asarray(results[c]["sexp"], dtype=np.float64)
        pexp = np.asarray(results[c]["pexp"], dtype=np.float64)
        # drop the self-similarity term from the softmax denominator, then
        # loss_i = log(sum_{j!=i} exp(sim/T)) - pos/T
        rows = np.log((S - sexp).T.reshape(-1)) - np.log(pexp.T.reshape(-1))
        per_row_loss[c * ROWS_PER_CORE : (c + 1) * ROWS_PER_CORE] = rows
    return np.float32(per_row_loss.mean())



# revision 7
# speedup vs baseline: 1.1076x; 1.1076x over previous
"""NT-Xent loss on 8 TRN2 NeuronCores.

Reference computes, for z = concat(z1, z2) (2N=8192 rows, D=256):
    zn  = z / max(||z||, eps)
    sim = (zn @ zn.T) / T, diag masked to -1e9
    loss = mean_i( logsumexp_j sim[i, j] - sim[i, pos(i)] ),  pos(i) = (i + N) % 2N

Sharding: 2N rows split into 8 blocks of 1024. Core c receives zn.T with
columns rotated left by c*1024, so its own rows sit at columns 0:1024 and
the positive partners at columns 4096:5120 on every core (compile-time
constant access patterns, identical program on all cores).

Per core: 32 tiles of [128 rows x 2048 cols].  TensorE computes each tile
with fp8(e4m3) DoubleRow matmuls (K=256 as 2 stacked K=128 planes, 2x rate).
The exp+rowsum over the tile is split across TWO engines working in
parallel from PSUM:
  - ScalarE (ACT): true exp via activation(Exp, scale, accum_out) - 17 tiles
  - VectorE (DVE): custom 8-stage op  X = (alpha*p + beta)^2 + gamma;
    out = X^8, accum_out = sum(X^8).  X is the deg-2 Taylor of
    exp(logit/8), so X^8 ~ exp(logit) (loss rel-err ~1e-4) - 15 tiles
The self-similarity diagonal is killed before exp by accumulating
-784 (= logit -43.75) onto it with a tiny eye-matmul into PSUM, so
exp underflows to ~0 and no host-side subtraction is needed (those
tiles always go to the ACT engine; the DVE approximation is only valid
on the off-diagonal logit range).  The positive logit is read directly
from the PSUM diagonal of column-block 2 via gpsimd indirect_copy
(exact, no exp round-trip).  Host: loss = mean(log(S) - pos*lam).
"""

import math
import sys

if "/opt/trn_rl_repo" not in sys.path:
    sys.path.insert(0, "/opt/trn_rl_repo")

from operator import add as _operator_add

import ml_dtypes
import numpy as np

import concourse.bass as bass
import concourse.mybir as mybir
import concourse.tile as tile
from concourse import bacc
from concourse.bass_utils import run_bass_kernel_spmd

N = 4096
D = 256
TWO_N = 2 * N          # 8192
TEMPERATURE = 0.07
EPS = 1e-8
N_CORES = 8
ROWS_PER_CORE = TWO_N // N_CORES   # 1024
M_TILES = ROWS_PER_CORE // 128     # 8 row-tiles of 128
CB = 2048                          # psum tile / column-block width
N_CB = TWO_N // CB                 # 4 column blocks
POS_CB = N // CB                   # column block holding the positives (2)

FP8_SCALE = 16.0                   # zn elements ~N(0,1/256) -> ~N(0,1)
LAM = 1.0 / (256.0 * TEMPERATURE)  # psum (=256*cos) -> logit
MU = LAM / 8.0                     # DVE computes exp(logit/8)^8
ALPHA = MU / math.sqrt(2.0)
BETA = math.sqrt(0.5)
GAMMA = 0.5
EYE_A = 28.0                       # diag pre-add: 28*-28 = -784 -> logit -43.75
EYE_B = -28.0

# engine per (m, cb): 'A' = ScalarE true exp, 'V' = DVE X^8 approx.
# cb 0 (self-diag) must be 'A'.  17 A / 15 V balances measured rates.
ASSIGN = {}
for _m in range(M_TILES):
    for _cb in range(N_CB):
        ASSIGN[(_m, _cb)] = "A" if _cb in (0, 2) else "V"
ASSIGN[(0, 3)] = "A"  # 17th ACT tile

_cached = {}


def _register_exp8_op():
    """Register the custom DVE op NTX_EXP8_REDUCE in concourse.dve_ops.OPS
    (the documented extension point; sha computed here so the pin check
    passes).  Idempotent."""
    import concourse.dve_ops as dve_ops
    from concourse.dve_spec import Spec, Src0, C0, C1, C2, Zero, sq, lower
    from concourse.dve_spec import _has_src1
    from concourse.dve_uop import DveOpSpec

    name = "NTX_EXP8_REDUCE"
    for op in dve_ops.OPS:
        if op.name == name:
            return op

    def _ref(in0, in1, s0, s1, imm2):
        x = (in0.astype(np.float32) * np.float32(s0) + np.float32(s1)) ** 2 + np.float32(imm2)
        b = ((x ** 2) ** 2) ** 2
        b = b.astype(np.float32)
        return b, b.reshape(b.shape[0], -1).sum(axis=-1, keepdims=True)

    body = sq(sq(sq(sq(Src0 * C0 + C1) + C2)))
    spec = Spec(body=body, accum=_operator_add, accum_init=Zero, reference=_ref)

    row = dve_ops._CUSTOM_DVE_ROW_BASE + len(dve_ops.OPS)
    shas = {}
    for ver in ("v3", "v4"):
        s = DveOpSpec(name=name, opcode=row, uops=lower(spec, ver=ver),
                      rd1_en=_has_src1(spec))
        shas[ver] = s.sha(ver)
    op = dve_ops.DveOp(name, spec, subdim=False, uops_sha=shas)
    dve_ops.OPS.append(op)
    dve_ops._SUB_OPCODE_FOR_NAME[name] = row
    return op


def _build_bass(m_tiles=M_TILES):
    f32 = mybir.dt.float32
    bf16 = mybir.dt.bfloat16
    fp8 = mybir.dt.float8e4
    exp8_op = _register_exp8_op()
    nc = bacc.Bacc("TRN2", target_bir_lowering=False, debug=False)

    znt = nc.declare_dram_parameter("znt", [D, TWO_N], fp8, isOutput=False)
    eye_a = nc.declare_dram_parameter("eye_a", [128, 128], fp8, isOutput=False)
    eye_d = nc.declare_dram_parameter("eye_d", [128, 4 * 512], fp8, isOutput=False)
    idx16 = nc.declare_dram_parameter("idx16", [128, 1], mybir.dt.uint16, isOutput=False)
    acc_out = nc.declare_dram_parameter("acc", [128, m_tiles * N_CB], f32, isOutput=True)
    pos_out = nc.declare_dram_parameter(
        "pos", [128, m_tiles], mybir.dt.bfloat16, isOutput=True
    )

    with tile.TileContext(nc) as tc:
        with (
            tc.tile_pool(name="zchunks", bufs=1) as zpool,
            tc.tile_pool(name="consts", bufs=1) as cpool,
            tc.tile_pool(name="stats", bufs=1) as spool,
            tc.tile_pool(name="aout", bufs=2) as apool,
            tc.tile_pool(name="vout", bufs=2) as vpool,
            tc.tile_pool(name="psum", bufs=2, space=bass.MemorySpace.PSUM) as ppool,
        ):
            # --- consts + ACT Exp-table preload overlap the input DMA ---
            warm = cpool.tile([128, 1], f32, tag="warm")
            nc.gpsimd.memset(warm[:], 0.0)
            nc.scalar.activation(
                out=warm[:], in_=warm[:],
                func=mybir.ActivationFunctionType.Exp, bias=0.0, scale=1.0,
            )

            eye_a_t = cpool.tile([128, 128], fp8, tag="eye_a")
            eye_d_t = cpool.tile([128, 4 * 512], fp8, tag="eye_d")
            idx_t = cpool.tile([128, 1], mybir.dt.uint16, tag="idx16")
            nc.gpsimd.dma_start(eye_a_t[:], eye_a[:])
            nc.gpsimd.dma_start(eye_d_t[:], eye_d[:])

            # znt chunks as [128, 2, CB]: plane k = contraction rows
            # k*128:(k+1)*128 (DoubleRow matmul layout).  Plane 0 via sync
            # queue, plane 1 via gpsimd queue (parallel descriptor issue).
            zt = []
            for cb in range(N_CB):
                t = zpool.tile([128, 2, CB], fp8, tag=f"z_{cb}")
                zt.append(t)
            for cb in range(N_CB):
                nc.sync.dma_start(
                    zt[cb][:, 0, :], znt[0:128, cb * CB : (cb + 1) * CB]
                )
                nc.gpsimd.dma_start(
                    zt[cb][:, 1, :], znt[128:256, cb * CB : (cb + 1) * CB]
                )
                if cb == 0:
                    # idx16 needed only from the first POS_CB tile on
                    nc.gpsimd.dma_start(idx_t[:], idx16[:])

            acc_t = spool.tile([128, m_tiles * N_CB], f32, tag="acc")
            pos_t = spool.tile([128, m_tiles], bf16, tag="pos")

            dr = mybir.MatmulPerfMode.DoubleRow
            for m in range(m_tiles):
                moff = m * 128
                for cb in range(N_CB):
                    ps = ppool.tile([128, CB], f32, tag="ps")
                    nn_diag = m // 4 if cb == 0 else -1
                    for nn in range(CB // 512):
                        dst = ps[:, nn * 512 : (nn + 1) * 512]
                        rhs = zt[cb][:, :, nn * 512 : (nn + 1) * 512]
                        lhsT = zt[0][:, :, moff : moff + 128]
                        if nn == nn_diag:
                            # pre-accumulate -784 onto the self-similarity
                            # diagonal so exp() underflows to 0
                            off = m % 4
                            nc.tensor.matmul(
                                dst,
                                lhsT=eye_a_t[:],
                                rhs=eye_d_t[:, off * 512 : (off + 1) * 512],
                                start=True,
                                stop=False,
                            )
                            nc.tensor.matmul(
                                dst, lhsT=lhsT, rhs=rhs,
                                start=False, stop=True, perf_mode=dr,
                            )
                        else:
                            nc.tensor.matmul(
                                dst, lhsT=lhsT, rhs=rhs,
                                start=True, stop=True, perf_mode=dr,
                            )
                    col = m * N_CB + cb
                    if ASSIGN[(m, cb)] == "A":
                        ex = apool.tile([128, CB], bf16, tag="aex")
                        nc.scalar.activation(
                            out=ex[:],
                            in_=ps[:],
                            func=mybir.ActivationFunctionType.Exp,
                            bias=0.0,
                            scale=LAM,
                            accum_out=acc_t[:, col : col + 1],
                        )
                    else:
                        ex = vpool.tile([128, CB], bf16, tag="vex")
                        nc.vector._custom_dve(
                            exp8_op,
                            out=ex[:],
                            in0=ps[:],
                            s0=ALPHA,
                            s1=BETA,
                            imm2=GAMMA,
                            accum_out=acc_t[:, col : col + 1],
                        )
                    if cb == POS_CB:
                        # positive-pair exp = diagonal of the exp'd SBUF
                        # tile (gpsimd cannot read PSUM); host takes log()
                        nc.gpsimd.indirect_copy(
                            out=pos_t[:, m : m + 1],
                            data=ex[:, moff : moff + 128],
                            idxs=idx_t[:],
                            i_know_ap_gather_is_preferred=True,
                        )

            nc.sync.dma_start(acc_out[:], acc_t[:])
            nc.sync.dma_start(pos_out[:], pos_t[:])

    nc.compile()
    return nc


def _prepare_inputs(z1, z2):
    z = np.concatenate([np.asarray(z1), np.asarray(z2)], axis=0).astype(np.float32)
    norms = np.maximum(np.sqrt((z.astype(np.float64) ** 2).sum(-1)), EPS)
    zn = (z / norms[:, None]).astype(np.float32)
    znq = (FP8_SCALE * zn).astype(ml_dtypes.float8_e4m3fn)
    znt = np.ascontiguousarray(znq.T)  # [D, 2N]
    eye_a = (EYE_A * np.eye(128, dtype=np.float32)).astype(ml_dtypes.float8_e4m3fn)
    eye_d = np.zeros((128, 4 * 512), dtype=np.float32)
    ll = np.arange(128)
    for off in range(4):
        eye_d[ll, off * 512 + off * 128 + ll] = EYE_B
    eye_d = eye_d.astype(ml_dtypes.float8_e4m3fn)
    idx16 = np.arange(128, dtype=np.uint16).reshape(128, 1)
    in_maps = []
    for c in range(N_CORES):
        znt_c = np.ascontiguousarray(np.roll(znt, -c * ROWS_PER_CORE, axis=1))
        in_maps.append(
            {"znt": znt_c, "eye_a": eye_a, "eye_d": eye_d, "idx16": idx16}
        )
    return in_maps


def kernel(z1, z2):
    if "nc" not in _cached:
        _cached["nc"] = _build_bass()
    nc = _cached["nc"]
    in_maps = _prepare_inputs(z1, z2)
    res = run_bass_kernel_spmd(nc, in_maps, core_ids=list(range(N_CORES)))
    results = res.results

    per_row_loss = np.zeros(TWO_N, dtype=np.float64)
    for c in range(N_CORES):
        # acc [128, 4*M]: element [l, m*4+cb] sums tile (m, cb) of rows
        # c*1024 + m*128 + l; diag already excluded on-device.
        acc = np.asarray(results[c]["acc"], dtype=np.float64)
        pos = np.asarray(results[c]["pos"], dtype=np.float64)  # exp-domain
        S = acc.reshape(128, M_TILES, N_CB).sum(-1)  # [128, M]
        rows = np.log(S.T.reshape(-1)) - np.log(pos).T.reshape(-1)
        per_row_loss[c * ROWS_PER_CORE : (c + 1) * ROWS_PER_CORE] = rows
    return np.float32(per_row_loss.mean())


# revision 12
# speedup vs baseline: 1.1233x; 1.0142x over previous
"""NT-Xent loss on 8 TRN2 NeuronCores.

Reference computes, for z = concat(z1, z2) (2N=8192 rows, D=256):
    zn  = z / max(||z||, eps)
    sim = (zn @ zn.T) / T, diag masked to -1e9
    loss = mean_i( logsumexp_j sim[i, j] - sim[i, pos(i)] ),  pos(i) = (i + N) % 2N

Sharding: 2N rows split into 8 blocks of 1024. Core c receives zn.T with
columns rotated left by c*1024, so its own rows sit at columns 0:1024 on
every core (compile-time constant access patterns, identical program on
all cores).

Per core: 32 tiles of [128 rows x 2048 cols].  TensorE computes each tile
with fp8(e4m3) DoubleRow matmuls (K=256 as 2 stacked K=128 planes).
The exp+rowsum over each tile runs on one of TWO engines in parallel:
  - ScalarE (ACT): true exp via activation(Exp, scale, accum_out)
  - VectorE (DVE): custom 8-stage op  X = (alpha*p + beta)^2 + gamma;
    out = X^8, accum_out = sum(X^8).  X is the deg-2 Taylor of
    exp(logit/8), so X^8 ~ exp(logit) (loss rel-err ~1e-4)
The self-similarity diagonal is killed before exp by accumulating
-784 (= logit -43.75) onto it with a small eye-matmul into PSUM, so exp
underflows to ~0 (those tiles always go to ACT; the DVE approximation
is only valid on the off-diagonal logit range).  The positive logits are
computed on the HOST from the same quantized fp8 rows (8192 length-256
dot products, exact).  Host: loss = mean(log(S) - pos_logit).
"""

import math
import sys

if "/opt/trn_rl_repo" not in sys.path:
    sys.path.insert(0, "/opt/trn_rl_repo")

from operator import add as _operator_add

import ml_dtypes
import numpy as np

import concourse.bass as bass
import concourse.mybir as mybir
import concourse.tile as tile
from concourse import bacc
from concourse.bass_utils import run_bass_kernel_spmd

N = 4096
D = 256
TWO_N = 2 * N          # 8192
TEMPERATURE = 0.07
EPS = 1e-8
N_CORES = 8
ROWS_PER_CORE = TWO_N // N_CORES   # 1024
M_TILES = ROWS_PER_CORE // 128     # 8 row-tiles of 128
CB = 2048                          # psum tile / column-block width
N_CB = TWO_N // CB                 # 4 column blocks

FP8_SCALE = 16.0                   # zn elements ~N(0,1/256) -> ~N(0,1)
LAM = 1.0 / (256.0 * TEMPERATURE)  # psum (=256*cos) -> logit
MU = LAM / 8.0                     # DVE computes exp(logit/8)^8
ALPHA = MU / math.sqrt(2.0)
BETA = math.sqrt(0.5)
GAMMA = 0.5
EYE_A = 28.0                       # diag pre-add: 28*-28 = -784 -> logit -43.75
EYE_B = -28.0

# engine per (m, cb): 'A' = ScalarE true exp, 'V' = DVE X^8 approx.
# cb 0 (self-diag) must be 'A'.  17 A / 15 V balances measured rates.
ASSIGN = {}
for _m in range(M_TILES):
    for _cb in range(N_CB):
        ASSIGN[(_m, _cb)] = "A" if _cb in (0, 2) else "V"
ASSIGN[(0, 3)] = "A"  # 17th ACT tile

_cached = {}


def _register_exp8_op():
    """Register the custom DVE op NTX_EXP8_REDUCE in concourse.dve_ops.OPS
    (the documented extension point; sha computed here so the pin check
    passes).  Idempotent."""
    import concourse.dve_ops as dve_ops
    from concourse.dve_spec import Spec, Src0, C0, C1, C2, Zero, sq, lower
    from concourse.dve_spec import _has_src1
    from concourse.dve_uop import DveOpSpec

    name = "NTX_EXP8_REDUCE"
    for op in dve_ops.OPS:
        if op.name == name:
            return op

    def _ref(in0, in1, s0, s1, imm2):
        x = (in0.astype(np.float32) * np.float32(s0) + np.float32(s1)) ** 2 + np.float32(imm2)
        b = ((x ** 2) ** 2) ** 2
        b = b.astype(np.float32)
        return b, b.reshape(b.shape[0], -1).sum(axis=-1, keepdims=True)

    body = sq(sq(sq(sq(Src0 * C0 + C1) + C2)))
    spec = Spec(body=body, accum=_operator_add, accum_init=Zero, reference=_ref)

    row = dve_ops._CUSTOM_DVE_ROW_BASE + len(dve_ops.OPS)
    shas = {}
    for ver in ("v3", "v4"):
        s = DveOpSpec(name=name, opcode=row, uops=lower(spec, ver=ver),
                      rd1_en=_has_src1(spec))
        shas[ver] = s.sha(ver)
    op = dve_ops.DveOp(name, spec, subdim=False, uops_sha=shas)
    dve_ops.OPS.append(op)
    dve_ops._SUB_OPCODE_FOR_NAME[name] = row
    return op


def _build_bass(m_tiles=M_TILES):
    f32 = mybir.dt.float32
    bf16 = mybir.dt.bfloat16
    fp8 = mybir.dt.float8e4
    exp8_op = _register_exp8_op()
    nc = bacc.Bacc("TRN2", target_bir_lowering=False, debug=False)

    # cst layout: [:, 0:128] = +28*I (stationary eye), [:, 128:2176] = four
    # 512-col blocks with a -28 diagonal at offset off*128 (off = m%4)
    znt = nc.declare_dram_parameter("znt", [D, TWO_N], fp8, isOutput=False)
    cst = nc.declare_dram_parameter("cst", [128, 128 + 4 * 512], fp8, isOutput=False)
    acc_out = nc.declare_dram_parameter(
        "acc", [128, 2 * m_tiles * N_CB], f32, isOutput=True
    )

    with tile.TileContext(nc) as tc:
        with (
            tc.tile_pool(name="zchunks", bufs=1) as zpool,
            tc.tile_pool(name="consts", bufs=1) as cpool,
            tc.tile_pool(name="stats", bufs=1) as spool,
            tc.tile_pool(name="aout", bufs=2) as apool,
            tc.tile_pool(name="vout", bufs=2) as vpool,
            tc.tile_pool(name="psum", bufs=2, space=bass.MemorySpace.PSUM) as ppool,
        ):
            # --- consts + ACT Exp-table preload overlap the input DMA ---
            warm = cpool.tile([128, 1], f32, tag="warm")
            nc.gpsimd.memset(warm[:], 0.0)
            nc.scalar.activation(
                out=warm[:], in_=warm[:],
                func=mybir.ActivationFunctionType.Exp, bias=0.0, scale=1.0,
            )

            cst_t = cpool.tile([128, 128 + 4 * 512], fp8, tag="cst")
            nc.gpsimd.dma_start(cst_t[:], cst[:])
            eye_l = cst_t[:, 0:128]

            # znt chunks as [128, 2, CB]: plane k = contraction rows
            # k*128:(k+1)*128 (DoubleRow matmul layout).  Plane 0 via sync
            # queue, plane 1 via gpsimd queue (parallel descriptor issue).
            zt = []
            for cb in range(N_CB):
                t = zpool.tile([128, 2, CB], fp8, tag=f"z_{cb}")
                zt.append(t)
            for cb in range(N_CB):
                nc.sync.dma_start(
                    zt[cb][:, 0, :], znt[0:128, cb * CB : (cb + 1) * CB]
                )
                nc.gpsimd.dma_start(
                    zt[cb][:, 1, :], znt[128:256, cb * CB : (cb + 1) * CB]
                )

            acc_a_t = spool.tile([128, m_tiles * N_CB], f32, tag="acc_a")
            acc_v_t = spool.tile([128, m_tiles * N_CB], f32, tag="acc_v")

            # PE p-state warm-up: ~10 filler matmuls into the first psum
            # slot while the z-chunk DMA is still in flight, so the PE has
            # ramped up by the time real tiles start.  No reader; the pool
            # recycles the slot immediately.
            warm_ps = ppool.tile([128, CB], f32, tag="ps")
            for _ in range(10):
                nc.tensor.matmul(
                    warm_ps[:, 0:512],
                    lhsT=eye_l,
                    rhs=cst_t[:, 128 : 128 + 512],
                    start=True,
                    stop=True,
                )

            dr = mybir.MatmulPerfMode.DoubleRow
            for m in range(m_tiles):
                moff = m * 128
                for cb in range(N_CB):
                    ps = ppool.tile([128, CB], f32, tag="ps")
                    lhsT = zt[0][:, :, moff : moff + 128]
                    nn_d = m // 4 if cb == 0 else -1
                    for nn in range(CB // 512):
                        dst = ps[:, nn * 512 : (nn + 1) * 512]
                        rhs = zt[cb][:, :, nn * 512 : (nn + 1) * 512]
                        if nn == nn_d:
                            # pre-add -784 on the self-sim diagonal so
                            # exp() underflows to 0
                            off = m % 4
                            nc.tensor.matmul(
                                dst,
                                lhsT=eye_l,
                                rhs=cst_t[:, 128 + off * 512 : 128 + (off + 1) * 512],
                                start=True,
                                stop=False,
                            )
                            nc.tensor.matmul(
                                dst, lhsT=lhsT, rhs=rhs,
                                start=False, stop=True, perf_mode=dr,
                            )
                        else:
                            nc.tensor.matmul(
                                dst, lhsT=lhsT, rhs=rhs,
                                start=True, stop=True, perf_mode=dr,
                            )
                    col = m * N_CB + cb
                    if ASSIGN[(m, cb)] == "A":
                        ex = apool.tile([128, CB], bf16, tag="aex")
                        nc.scalar.activation(
                            out=ex[:],
                            in_=ps[:],
                            func=mybir.ActivationFunctionType.Exp,
                            bias=0.0,
                            scale=LAM,
                            accum_out=acc_a_t[:, col : col + 1],
                        )
                    else:
                        ex = vpool.tile([128, CB], bf16, tag="vex")
                        nc.vector._custom_dve(
                            exp8_op,
                            out=ex[:],
                            in0=ps[:],
                            s0=ALPHA,
                            s1=BETA,
                            imm2=GAMMA,
                            accum_out=acc_v_t[:, col : col + 1],
                        )

            nc.sync.dma_start(acc_out[:, 0 : m_tiles * N_CB], acc_a_t[:])
            nc.sync.dma_start(acc_out[:, m_tiles * N_CB :], acc_v_t[:])

    nc.compile()
    return nc


def _prepare_inputs(z1, z2):
    z = np.concatenate([np.asarray(z1), np.asarray(z2)], axis=0).astype(np.float32)
    norms = np.maximum(np.sqrt((z.astype(np.float64) ** 2).sum(-1)), EPS)
    zn = (z / norms[:, None]).astype(np.float32)
    znq = (FP8_SCALE * zn).astype(ml_dtypes.float8_e4m3fn)
    znt = np.ascontiguousarray(znq.T)  # [D, 2N]
    cst = np.zeros((128, 128 + 4 * 512), dtype=np.float32)
    ll = np.arange(128)
    cst[ll, ll] = EYE_A
    for off in range(4):
        cst[ll, 128 + off * 512 + off * 128 + ll] = EYE_B
    cst = cst.astype(ml_dtypes.float8_e4m3fn)
    in_maps = []
    for c in range(N_CORES):
        znt_c = np.ascontiguousarray(np.roll(znt, -c * ROWS_PER_CORE, axis=1))
        in_maps.append({"znt": znt_c, "cst": cst})
    # positive logits computed host-side from the same quantized rows
    zq = znq.astype(np.float32)
    pd = (zq[:N] * zq[N:]).sum(-1) * LAM          # [N]
    pos_logit = np.concatenate([pd, pd]).astype(np.float64)
    return in_maps, pos_logit


def kernel(z1, z2):
    if "nc" not in _cached:
        _cached["nc"] = _build_bass()
    nc = _cached["nc"]
    in_maps, pos_logit = _prepare_inputs(z1, z2)
    res = run_bass_kernel_spmd(nc, in_maps, core_ids=list(range(N_CORES)))
    results = res.results

    a_cols = np.array(
        [ASSIGN[(m, cb)] == "A" for m in range(M_TILES) for cb in range(N_CB)]
    )
    per_row_loss = np.zeros(TWO_N, dtype=np.float64)
    for c in range(N_CORES):
        # acc [128, 4*M]: element [l, m*4+cb] sums tile (m, cb) of rows
        # c*1024 + m*128 + l; diag already excluded on-device.
        acc_all = np.asarray(results[c]["acc"], dtype=np.float64)
        acc_a, acc_v = acc_all[:, : M_TILES * N_CB], acc_all[:, M_TILES * N_CB :]
        acc = np.where(a_cols[None, :], acc_a, acc_v)
        S = acc.reshape(128, M_TILES, N_CB).sum(-1)  # [128, M]
        rows = np.log(S.T.reshape(-1))
        per_row_loss[c * ROWS_PER_CORE : (c + 1) * ROWS_PER_CORE] = rows
    per_row_loss -= pos_logit
    return np.float32(per_row_loss.mean())
